# revision 5
# baseline (speedup 1.0000x reference)
"""Trainium2 Bass kernel for nn_KernelMultiHeadAttention (RBF-kernel attention).

Math (per batch b):
  ctx  = query @ q_weight.T + q_bias          [L, H]
  A    = weight @ weight.T (PSD)              [H, H]
  dist_ij = (ctx_i - ctx_j)^T A (ctx_i - ctx_j)
  scores  = exp(-0.5 * scale * dist)          scale = precision_inv_dis[0,0]^2
  out     = softmax(scores, axis=-1)

Kernel formulation: with z = W^T ctx (so dist_ij = ||z_i - z_j||^2):
  z^T = (W^T q_weight) @ query^T + W^T q_bias          [H, L]
  exponent_ij = scale*(z_i . z_j - 0.5||z_i||^2 - 0.5||z_j||^2)
The -0.5||z_j||^2 (free-axis) term rides as an extra contraction row in the
scores matmul (K = H+1 = 121); the -0.5*scale*||z_i||^2 (partition-axis) term
is the per-partition bias of the Exp activation, which also applies the
runtime `scale` multiplier. Softmax skips max-subtraction: scores are in
(0, 1] so exp(scores) is in [1, e] - no overflow possible.

Sharding: data-parallel over batch, core b <- batch b (8 cores, 8 batches).
"""
import numpy as np
from contextlib import ExitStack

import concourse.bass as bass
import concourse.mybir as mybir
import concourse.tile as tile
from concourse import bacc
from concourse import bass_utils
from concourse.masks import make_identity

B, L, D, H = 8, 2048, 1024, 120
N_CORES = 8
FP32 = mybir.dt.float32
AF = mybir.ActivationFunctionType

_cache = {}


def _build():
    nc = bacc.Bacc("TRN2", target_bir_lowering=False, debug=False,
                   num_devices=N_CORES)
    q_d = nc.dram_tensor("query", [L, D], FP32, kind="ExternalInput").ap()
    wq_d = nc.dram_tensor("q_weight", [H, D], FP32, kind="ExternalInput").ap()
    qb_d = nc.dram_tensor("q_bias", [H], FP32, kind="ExternalInput").ap()
    w_d = nc.dram_tensor("weight", [H, H], FP32, kind="ExternalInput").ap()
    pid_d = nc.dram_tensor("precision_inv_dis", [1, 1], FP32,
                           kind="ExternalInput").ap()
    out_d = nc.dram_tensor("out", [L, L], FP32, kind="ExternalOutput").ap()

    NI = L // 128          # 16 i-tiles
    NC_ = D // 128         # 8 d-chunks
    NJ = L // 512          # 4 j-chunks

    with tile.TileContext(nc) as tc:
        with ExitStack() as ctx:
            const = ctx.enter_context(tc.tile_pool(name="const", bufs=1))
            qin = ctx.enter_context(tc.tile_pool(name="qin", bufs=3))
            qt_pool = ctx.enter_context(tc.tile_pool(name="qt", bufs=1))
            epool = ctx.enter_context(tc.tile_pool(name="e", bufs=2))
            cols = ctx.enter_context(tc.tile_pool(name="cols", bufs=4))
            ps_big = ctx.enter_context(
                tc.tile_pool(name="psb", bufs=4, space="PSUM"))
            ps_sm = ctx.enter_context(
                tc.tile_pool(name="pss", bufs=2, space="PSUM"))

            # ---- constants / params ----
            ident = const.tile([128, 128], FP32, tag="ident")
            make_identity(nc, ident[:])

            w_sb = const.tile([H, H], FP32, tag="w")
            nc.sync.dma_start(w_sb[:], w_d[:])
            wq_sb = const.tile([H, D], FP32, tag="wq")
            nc.sync.dma_start(wq_sb[:], wq_d[:])
            qb_col = const.tile([H, 1], FP32, tag="qb")
            nc.sync.dma_start(
                qb_col[:],
                bass.AP(tensor=qb_d.tensor, offset=qb_d.offset,
                        ap=[[1, H], [1, 1]]))
            pid_b = const.tile([128, 1], FP32, tag="pid")
            nc.sync.dma_start(
                pid_b[:],
                bass.AP(tensor=pid_d.tensor, offset=pid_d.offset,
                        ap=[[0, 128], [1, 1]]))
            s_bcast = const.tile([128, 1], FP32, tag="sb")
            nc.vector.tensor_mul(s_bcast[:], pid_b[:], pid_b[:])
            s_neg_half = const.tile([128, 1], FP32, tag="snh")
            nc.scalar.mul(s_neg_half[:], s_bcast[:], -0.5)
            ones_col = const.tile([H, 1], FP32, tag="ones")
            nc.gpsimd.memset(ones_col[:], 1.0)
            neghalf_col = const.tile([H, 1], FP32, tag="negh")
            nc.gpsimd.memset(neghalf_col[:], -0.5)

            # ---- M = W^T @ Wq  [H, D];  wb = W^T @ q_bias [H, 1] ----
            m_sb = const.tile([H, D], FP32, tag="m")
            for c in range(D // 512):
                m_ps = ps_big.tile([128, 512], FP32, tag="big")
                nc.tensor.matmul(m_ps[:H, :], w_sb[:], wq_sb[:, c*512:(c+1)*512],
                                 start=True, stop=True)
                nc.scalar.copy(m_sb[:, c*512:(c+1)*512], m_ps[:H, :])
            wb_ps = ps_sm.tile([128, 1], FP32, tag="sm")
            nc.tensor.matmul(wb_ps[:H, :], w_sb[:], qb_col[:],
                             start=True, stop=True)
            wb_col = const.tile([H, 1], FP32, tag="wb")
            nc.vector.tensor_copy(wb_col[:], wb_ps[:H, :])

            # M^T chunks: [128, H] x NC_   (for z^T matmul lhsT)
            mt_sb = const.tile([128, NC_, H], FP32, tag="mt")
            for c in range(NC_):
                t_ps = ps_big.tile([128, 512], FP32, tag="big")
                nc.tensor.transpose(t_ps[:, :H], m_sb[:, c*128:(c+1)*128],
                                    ident[:H, :H])
                nc.vector.tensor_copy(mt_sb[:, c, :], t_ps[:, :H])

            # ---- transpose query: Qt[c] [128 d, L i] ----
            qt = [qt_pool.tile([128, L], FP32, tag=f"qt{c}", name=f"qt{c}")
                  for c in range(NC_)]
            for i in range(NI):
                q_tile = qin.tile([128, D], FP32, tag="qtile")
                nc.sync.dma_start(q_tile[:], q_d[i*128:(i+1)*128, :])
                for c in range(NC_):
                    tp = ps_big.tile([128, 512], FP32, tag="big")
                    nc.tensor.transpose(tp[:, :128],
                                        q_tile[:, c*128:(c+1)*128], ident[:])
                    if (i + c) % 2 == 0:
                        nc.scalar.copy(qt[c][:, i*128:(i+1)*128], tp[:, :128])
                    else:
                        nc.vector.tensor_copy(qt[c][:, i*128:(i+1)*128],
                                              tp[:, :128])

            # ---- z^T = M @ Q^T + wb  -> Zl rows 0:120 (+ones row),
            #      Zr rows 0:120 (+ -0.5*||z||^2 row) ----
            zl = const.tile([121, L], FP32, tag="zl")
            zr = const.tile([121, L], FP32, tag="zr")
            zsq = const.tile([H, L], FP32, tag="zsq")
            ones_row = const.tile([1, L], FP32, tag="onesrow")
            nc.gpsimd.memset(ones_row[:], 1.0)
            nc.sync.dma_start(zl[120:121, :], ones_row[:])
            nr_sb = const.tile([1, L], FP32, tag="nrow")
            for jc in range(NJ):
                sl = slice(jc*512, (jc+1)*512)
                z_ps = ps_big.tile([128, 512], FP32, tag="big")
                for c in range(NC_):
                    nc.tensor.matmul(z_ps[:H, :], mt_sb[:, c, :], qt[c][:, sl],
                                     start=(c == 0), stop=(c == NC_ - 1))
                nc.scalar.activation(zl[:H, sl], z_ps[:H, :], AF.Identity,
                                     bias=wb_col[:], scale=1.0)
                nc.vector.tensor_copy(zr[:H, sl], zl[:H, sl])
                nc.vector.tensor_mul(zsq[:, sl], zl[:H, sl], zl[:H, sl])
                nr_ps = ps_sm.tile([128, 512], FP32, tag="sm")
                nc.tensor.matmul(nr_ps[:1, :], neghalf_col[:], zsq[:, sl],
                                 start=True, stop=True)
                nc.scalar.copy(nr_sb[:1, sl], nr_ps[:1, :])
            nc.sync.dma_start(zr[120:121, :], nr_sb[:])

            # ---- scores + double-exp softmax, streamed over i-tiles ----
            for i in range(NI):
                isl = bass.ts(i, 128)
                ncol_ps = ps_sm.tile([128, 512], FP32, tag="sm")
                nc.tensor.matmul(ncol_ps[:, :1], zsq[:, isl], ones_col[:],
                                 start=True, stop=True)
                b_col = cols.tile([128, 1], FP32, tag="bcol")
                # b_col = -0.5 * scale * ||z_i||^2
                nc.scalar.mul(b_col[:], ncol_ps[:, :1], s_neg_half[:])

                e1 = epool.tile([128, L], FP32, tag="e1")
                e2 = epool.tile([128, L], FP32, tag="e2")
                for jc in range(NJ):
                    sl = slice(jc*512, (jc+1)*512)
                    q_ps = ps_big.tile([128, 512], FP32, tag="big")
                    nc.tensor.matmul(q_ps[:], zl[:, isl], zr[:, sl],
                                     start=True, stop=True)
                    # e1 = exp(scale * (z_i.z_j - 0.5||z_j||^2) - 0.5*scale*||z_i||^2)
                    nc.scalar.activation(e1[:, sl], q_ps[:], AF.Exp,
                                         bias=b_col[:], scale=s_bcast[:])
                    # e2 = exp(e1)   (softmax numerator; no max needed)
                    nc.scalar.activation(e2[:, sl], e1[:, sl], AF.Exp)

                rsum = cols.tile([128, 1], FP32, tag="rsum")
                nc.vector.reduce_sum(rsum[:], e2[:], axis=mybir.AxisListType.X)
                rinv = cols.tile([128, 1], FP32, tag="rinv")
                nc.vector.reciprocal(rinv[:], rsum[:])
                nc.vector.tensor_scalar_mul(e2[:], e2[:], rinv[:])
                nc.sync.dma_start(out_d[i*128:(i+1)*128, :], e2[:])

    nc.compile()
    return nc


def kernel(query, key=None, q_weight=None, q_bias=None, weight=None,
           precision_inv_dis=None, **_ignored):
    query = np.ascontiguousarray(np.asarray(query, dtype=np.float32))
    q_weight = np.ascontiguousarray(np.asarray(q_weight, dtype=np.float32))
    q_bias = np.ascontiguousarray(np.asarray(q_bias, dtype=np.float32))
    weight = np.ascontiguousarray(np.asarray(weight, dtype=np.float32))
    precision_inv_dis = np.ascontiguousarray(
        np.asarray(precision_inv_dis, dtype=np.float32))

    if "nc" not in _cache:
        _cache["nc"] = _build()
    nc = _cache["nc"]

    in_maps = []
    for b in range(N_CORES):
        in_maps.append({
            "query": query[b],
            "q_weight": q_weight,
            "q_bias": q_bias,
            "weight": weight,
            "precision_inv_dis": precision_inv_dis,
        })
    res = bass_utils.run_bass_kernel_spmd(
        nc, in_maps, core_ids=list(range(N_CORES)))
    out = np.empty((B, L, L), dtype=np.float32)
    for b in range(N_CORES):
        out[b] = res.results[b]["out"]
    return out
